# revision 19
# baseline (speedup 1.0000x reference)
"""Trainium2 Bass kernel for nn_DecoderRNN (GRU decoder w/ teacher forcing).

Computation: 64-step GRU (B=32, H=1024) + output projection to V=32000
+ log_softmax.  Distribution over 8 NeuronCores:
  - input-gate GEMM (GI = relu(emb[tok]) @ W_ih.T + biases) sharded over
    time (8 steps/core) and AllGathered;
  - the sequential GRU recurrence replicated on every core, with the
    [32,3072] per-step gate GEMM packed 4-wide into PE column groups
    (quad layout: partition = 32*jh + b) for full array utilization;
  - output projection vocab-sharded (4000 cols/core) from a transposed
    hidden-state buffer produced by per-step PE transposes;
  - softmax via exp(logits)+row-sum fused on ScalarE, one 8KB AllReduce
    of the local sums, then log(exp*1/S) finalization.

Numerics: bf16 matmul operands with fp32 PSUM accumulation; gate math in
fp32; exp values stored bf16.
"""

import sys

for _p in ("/opt/trn_rl_repo",):
    if _p not in sys.path:
        sys.path.insert(0, _p)

import numpy as np
import ml_dtypes

import concourse.bass as bass
import concourse.mybir as mybir
from concourse.tile import TileContext
from concourse.masks import make_identity

# ---------------------------------------------------------------------------
# Workaround for this container's walrus build: the TileContext final drain
# may carry only ONE sync-wait command.  Spread extra waits across
# single-wait SP NoOps emitted just before the drain.
# ---------------------------------------------------------------------------
try:
    from concourse.tile import ScopedClock
except ImportError:  # pragma: no cover
    from bass_rust import ScopedClock  # type: ignore

_MAX_DRAIN_WAITS = 1


def _spread_multi_waits(nc):
    """Leave at most one sync-wait per instruction; hoist extras onto
    single-wait same-engine NoOps inserted immediately before."""
    for bb in nc.main_func.blocks:
        if not any(
            ins.sync_info and ins.sync_info.on_wait and len(ins.sync_info.on_wait) > 1
            for ins in bb.instructions
        ):
            continue
        new_list = []
        for ins in bb.instructions:
            si = ins.sync_info
            if si is not None and si.on_wait and len(si.on_wait) > _MAX_DRAIN_WAITS:
                waits = list(si.on_wait)
                del si.on_wait[: -_MAX_DRAIN_WAITS]
                for w in waits[:-_MAX_DRAIN_WAITS]:
                    nop = mybir.InstNoOp(
                        name=f"I-mw-{nc.next_id()}",
                        sync_info=mybir.SyncInfo(on_wait=[w], on_update=[]),
                        engine=ins.engine,
                        bass_nofuse=True,
                    )
                    nc.register_instruction(nop)
                    new_list.append(nop)
            new_list.append(ins)
        bb.instructions[:] = new_list


def _patched_drain_and_barrier(self, tick_clock, wait_clock):
    nc = self.nc
    drain_inst = nc.sync.drain()
    wait_clock.add_sem_waits(
        drain_inst.ins, ScopedClock({None: tick_clock.global_clock})
    )
    nc.all_engine_barrier()
    assert self.sems is not None
    popped = nc._tile_sem_poison_stack.pop()
    assert popped is self._sem_poison
    nc.clear_and_free_semaphores(list(self.sems.allocated().values()))
    nc.all_engine_barrier()
    _spread_multi_waits(nc)


TileContext._drain_and_barrier = _patched_drain_and_barrier

# ---------------------------------------------------------------------------

F32 = mybir.dt.float32
BF16 = mybir.dt.bfloat16
AF = mybir.ActivationFunctionType

N_CORES = 8
B, T, H, V = 32, 64, 1024, 32000
G = 3 * H  # 3072
VC = V // N_CORES  # 4000 vocab columns per core
SOS = 1
TPC = T // N_CORES  # 8 time steps per core for the GI phase
TAU_PC = TPC // 4  # row tiles per core in the GI phase
XW = TPC * 32  # x columns per core
NTAU = T // 4  # 16 row tiles of 128 = (4 steps x 32 batch)
KC = H // 128  # 8 contraction chunks
NV8 = 8  # vocab sub-blocks per core
VB = VC // NV8  # 500
CC_ADDR_SPACE = "Shared"  # collective outputs; sim with <4 cores needs Local


def _build() -> bass.Bass:
    nc = bass.Bass()

    xt_d = nc.dram_tensor("xt", [H, XW], BF16, kind="ExternalInput")
    wih_d = nc.dram_tensor("wih", [H + 1, G], BF16, kind="ExternalInput")
    whh_d = nc.dram_tensor("whh", [H + 1, G], BF16, kind="ExternalInput")
    wout_d = nc.dram_tensor("wout", [H + 1, VC], BF16, kind="ExternalInput")
    h0q_d = nc.dram_tensor("h0q", [128, 256], F32, kind="ExternalInput")
    h0te_d = nc.dram_tensor("h0te", [128, 128], BF16, kind="ExternalInput")
    bhn_d = nc.dram_tensor("bhn", [128, 256], F32, kind="ExternalInput")
    h0to_d = nc.dram_tensor("h0to", [128, 128], BF16, kind="ExternalInput")

    out_lp_d = nc.dram_tensor("out_lp", [NTAU, 128, VC], F32, kind="ExternalOutput")
    out_h_d = nc.dram_tensor("out_h", [128, 256], F32, kind="ExternalOutput")

    rg = [list(range(N_CORES))]

    with TileContext(nc) as tc:
        with (
            tc.tile_pool(name="persist", bufs=1) as pp,
            tc.tile_pool(name="dram", bufs=1, space="DRAM") as dram,
        ):
            ident = pp.tile([128, 128], F32)
            make_identity(nc, ident[:])
            ones_sb = pp.tile([1, 256], BF16)
            nc.gpsimd.memset(ones_sb[:], 1.0)
            bhn_sb = pp.tile([128, 256], F32)
            nc.sync.dma_start(out=bhn_sb[:], in_=bhn_d[:])

            HT_e = pp.tile([128, T * 128], BF16)
            HT_o = pp.tile([128, T * 128], BF16)
            h0te_sb = pp.tile([128, 128], BF16)
            h0to_sb = pp.tile([128, 128], BF16)
            Sacc = pp.tile([128, NTAU * NV8], F32)
            Sloc = pp.tile([128, NTAU], F32)
            Ssum = pp.tile([128, NTAU], F32)
            recipS = pp.tile([128, NTAU], F32)

            gi_local = dram.tile([TAU_PC, 128, G], F32)
            gi_all = dram.tile([NTAU, 128, G], F32, addr_space=CC_ADDR_SPACE)
            cc_in = dram.tile([128, NTAU], F32)
            cc_out = dram.tile([128, NTAU], F32, addr_space=CC_ADDR_SPACE)

            # ---- input loads ----
            nc.sync.dma_start(out=h0te_sb[:], in_=h0te_d[:])
            nc.sync.dma_start(out=h0to_sb[:], in_=h0to_d[:])

            # whh lives in its own pool so it frees before phase 3
            whh_pool = tc.tile_pool(name="whhp", bufs=1)
            whp = whh_pool.__enter__()
            whh_sb = whp.tile([128, KC * G], BF16)
            for kc in range(KC):
                nc.sync.dma_start(
                    out=whh_sb[:, G * kc : G * (kc + 1)],
                    in_=whh_d[128 * kc : 128 * (kc + 1), :],
                )
            # ---- phase 1: GI = relu(x) @ W_ih'.T + biases (2 row tiles) ----
            with (
                tc.tile_pool(name="p1", bufs=1) as p1,
                tc.tile_pool(name="ps1", bufs=1, space="PSUM") as ps1,
            ):
                wih_sb = p1.tile([128, KC * G], BF16)
                wih_aug = p1.tile([1, G], BF16)
                for kc in range(KC):
                    nc.sync.dma_start(
                        out=wih_sb[:, G * kc : G * (kc + 1)],
                        in_=wih_d[128 * kc : 128 * (kc + 1), :],
                    )
                nc.sync.dma_start(out=wih_aug[:], in_=wih_d[H : H + 1, :])
                xt_raw = p1.tile([128, KC * XW], BF16)
                xtr = p1.tile([128, KC * XW], BF16)
                for kc in range(KC):
                    nc.sync.dma_start(
                        out=xt_raw[:, XW * kc : XW * (kc + 1)],
                        in_=xt_d[128 * kc : 128 * (kc + 1), :],
                    )
                nc.scalar.activation(xtr[:], xt_raw[:], AF.Relu)

                for tl in range(TAU_PC):
                    gps = ps1.tile([128, G], F32, tag="gip")
                    gsb = p1.tile([128, G], F32, tag="gsb")
                    for kc in range(KC):
                        lhsT = xtr[:, XW * kc + 128 * tl : XW * kc + 128 * tl + 128]
                        for n in range(G // 512):
                            nc.tensor.matmul(
                                gps[:, 512 * n : 512 * (n + 1)],
                                lhsT,
                                wih_sb[:, G * kc + 512 * n : G * kc + 512 * (n + 1)],
                                start=(kc == 0),
                                stop=False,
                            )
                    for n in range(G // 512):
                        nc.tensor.matmul(
                            gps[:, 512 * n : 512 * (n + 1)],
                            ones_sb[:, 0:128],
                            wih_aug[:, 512 * n : 512 * (n + 1)],
                            start=False,
                            stop=True,
                        )
                    nc.vector.tensor_copy(out=gsb[:], in_=gps[:])
                    nc.sync.dma_start(out=gi_local[tl], in_=gsb[:])

                nc.gpsimd.collective_compute(
                    "AllGather",
                    mybir.AluOpType.bypass,
                    replica_groups=rg,
                    ins=[gi_local.opt()],
                    outs=[gi_all.opt()],
                )

            # ---- phase 2: GRU recurrence, 64 steps ----
            with (
                tc.tile_pool(name="p2s", bufs=3) as p2s,
                tc.tile_pool(name="p2b", bufs=2) as p2b,
                tc.tile_pool(name="ps2", bufs=1, space="PSUM") as ps2,
            ):
                hq_prev = p2b.tile([128, 256], F32, tag="hq")
                nc.sync.dma_start(out=hq_prev[:], in_=h0q_d[:])

                for t in range(T):
                    tau, t4 = divmod(t, 4)
                    gi_t = p2s.tile([128, 768], F32, tag="gi")
                    src = (
                        gi_all[tau, 32 * t4 : 32 * t4 + 32, :]
                        .rearrange("b (j f) -> b j f", j=4)
                        .transpose((1, 0, 2))
                    )
                    nc.sync.dma_start(out=gi_t[:], in_=src)

                    P_rz = ps2.tile([128, 512], F32, tag="grz")
                    P_n = ps2.tile([128, 512], F32, tag="gn")
                    for kc in (0, 2, 4, 6, 1, 3, 5, 7):
                        if t == 0:
                            lsrc = h0te_sb if kc % 2 == 0 else h0to_sb
                            col = 32 * (kc // 2)
                        else:
                            lsrc = HT_e if kc % 2 == 0 else HT_o
                            col = 32 * T * (kc // 2) + 32 * (t - 1)
                        lhsT = lsrc[:, col : col + 32]
                        for jh in range(4):
                            base = G * kc + 768 * jh
                            nc.tensor.matmul(
                                P_rz[32 * jh : 32 * jh + 32, 0:512],
                                lhsT,
                                whh_sb[:, base : base + 512],
                                start=(kc == 0),
                                stop=(kc == 7),
                                tile_position=(0, 32 * jh),
                                skip_group_check=True,
                            )
                            nc.tensor.matmul(
                                P_n[32 * jh : 32 * jh + 32, 0:256],
                                lhsT,
                                whh_sb[:, base + 512 : base + 768],
                                start=(kc == 0),
                                stop=(kc == 7),
                                tile_position=(0, 32 * jh),
                                skip_group_check=True,
                            )

                    rzpre = p2b.tile([128, 512], F32, tag="rzpre")
                    nc.vector.tensor_add(
                        out=rzpre[:], in0=P_rz[:], in1=gi_t[:, 0:512]
                    )
                    rz = p2b.tile([128, 512], F32, tag="rz")
                    nc.scalar.activation(rz[:], rzpre[:], AF.Sigmoid)
                    pnb = p2b.tile([128, 256], F32, tag="pnb")
                    nc.vector.tensor_add(out=pnb[:], in0=P_n[:, 0:256], in1=bhn_sb[:])
                    t1 = p2b.tile([128, 256], F32, tag="t1")
                    nc.vector.tensor_mul(out=t1[:], in0=rz[:, 0:256], in1=pnb[:])
                    t2 = p2b.tile([128, 256], F32, tag="t2")
                    nc.vector.tensor_add(out=t2[:], in0=t1[:], in1=gi_t[:, 512:768])
                    nt = p2b.tile([128, 256], F32, tag="nt")
                    nc.scalar.activation(nt[:], t2[:], AF.Tanh)
                    d = p2b.tile([128, 256], F32, tag="d")
                    nc.vector.tensor_sub(out=d[:], in0=hq_prev[:], in1=nt[:])
                    zd = p2b.tile([128, 256], F32, tag="zd")
                    nc.vector.tensor_mul(out=zd[:], in0=rz[:, 256:512], in1=d[:])
                    hq_new = p2b.tile([128, 256], F32, tag="hq")
                    nc.vector.tensor_add(out=hq_new[:], in0=nt[:], in1=zd[:])

                    tp0 = ps2.tile([128, 512], F32, tag="tp0")
                    nc.tensor.transpose(tp0[:, 0:128], hq_new[:, 0:128], ident[:])
                    tp1 = ps2.tile([128, 512], F32, tag="tp1")
                    nc.tensor.transpose(tp1[:, 0:128], hq_new[:, 128:256], ident[:])
                    hte_w = HT_e[:].rearrange("p (j t b) -> p j t b", j=4, b=32)
                    hto_w = HT_o[:].rearrange("p (j t b) -> p j t b", j=4, b=32)
                    nc.vector.tensor_copy(
                        out=hte_w[:, :, t, :],
                        in_=tp0[:, 0:128].rearrange("p (j b) -> p j b", j=4),
                    )
                    nc.scalar.activation(
                        hto_w[:, :, t, :],
                        tp1[:, 0:128].rearrange("p (j b) -> p j b", j=4),
                        AF.Copy,
                    )
                    hq_prev = hq_new

                nc.sync.dma_start(out=out_h_d[:], in_=hq_prev[:])

            whh_pool.__exit__(None, None, None)

            # ---- phase 3: logits + softmax ----
            with (
                tc.tile_pool(name="p3", bufs=1) as p3,
                tc.tile_pool(name="p3w", bufs=2) as p3w,
                tc.tile_pool(name="p3o", bufs=3) as p3o,
                tc.tile_pool(name="ps3", bufs=1, space="PSUM") as ps3,
            ):
                exp_sb = p3.tile([128, NTAU * VC], BF16)

                for v8 in range(NV8):
                    w8 = p3w.tile([128, KC * VB], BF16, tag="w8")
                    a8 = p3w.tile([1, VB], BF16, tag="a8")
                    for kc in range(KC):
                        nc.sync.dma_start(
                            out=w8[:, VB * kc : VB * (kc + 1)],
                            in_=wout_d[
                                128 * kc : 128 * (kc + 1), VB * v8 : VB * (v8 + 1)
                            ],
                        )
                    nc.sync.dma_start(
                        out=a8[:], in_=wout_d[H : H + 1, VB * v8 : VB * (v8 + 1)]
                    )
                    for tau in range(NTAU):
                        pl = ps3.tile([128, 512], F32, tag=f"lg{tau % 4}")
                        for kc in range(KC):
                            hv = HT_e if kc % 2 == 0 else HT_o
                            j = kc // 2
                            lhsT = hv[:, 32 * T * j + 128 * tau : 32 * T * j + 128 * tau + 128]
                            nc.tensor.matmul(
                                pl[:, 0:VB],
                                lhsT,
                                w8[:, VB * kc : VB * (kc + 1)],
                                start=(kc == 0),
                                stop=False,
                            )
                        nc.tensor.matmul(
                            pl[:, 0:VB], ones_sb[:, 0:128], a8[:], start=False, stop=True
                        )
                        col = VC * tau + VB * v8
                        nc.scalar.activation(
                            exp_sb[:, col : col + VB],
                            pl[:, 0:VB],
                            AF.Exp,
                            accum_out=Sacc[:, 8 * tau + v8 : 8 * tau + v8 + 1],
                        )

                nc.vector.tensor_reduce(
                    out=Sloc[:].rearrange("p (t o) -> p t o", o=1),
                    in_=Sacc[:].rearrange("p (t v) -> p t v", v=8),
                    axis=mybir.AxisListType.X,
                    op=mybir.AluOpType.add,
                )
                nc.sync.dma_start(out=cc_in[:], in_=Sloc[:])
                nc.gpsimd.collective_compute(
                    "AllReduce",
                    mybir.AluOpType.add,
                    replica_groups=rg,
                    ins=[cc_in.opt()],
                    outs=[cc_out.opt()],
                )
                nc.sync.dma_start(out=Ssum[:], in_=cc_out[:])
                nc.vector.reciprocal(recipS[:], Ssum[:])

                for tau in range(NTAU):
                    for v8 in range(NV8):
                        ot = p3o.tile([128, VB], F32, tag="ot")
                        col = VC * tau + VB * v8
                        nc.scalar.activation(
                            ot[:],
                            exp_sb[:, col : col + VB],
                            AF.Ln,
                            scale=recipS[:, tau : tau + 1],
                        )
                        nc.sync.dma_start(
                            out=out_lp_d[tau, :, VB * v8 : VB * (v8 + 1)], in_=ot[:]
                        )

    return nc


_RUNNER = None


def _get_runner():
    """Build the Bass program once and return a cached PJRT executor."""
    global _RUNNER
    if _RUNNER is not None:
        return _RUNNER

    import jax
    import jax.numpy as jnp  # noqa: F401
    from jax.sharding import Mesh, PartitionSpec
    from jax.experimental.shard_map import shard_map
    from concourse import bass2jax

    nc = _build()
    bass2jax.install_neuronx_cc_hook()

    partition_name = nc.partition_id_tensor.name if nc.partition_id_tensor else None
    in_names, out_names, out_avals, zero_outs = [], [], [], []
    for alloc in nc.m.functions[0].allocations:
        if not isinstance(alloc, mybir.MemoryLocationSet):
            continue
        name = alloc.memorylocations[0].name
        if alloc.kind == "ExternalInput":
            if name != partition_name:
                in_names.append(name)
        elif alloc.kind == "ExternalOutput":
            shape = tuple(alloc.tensor_shape)
            dtype = mybir.dt.np(alloc.dtype)
            out_names.append(name)
            out_avals.append(jax.core.ShapedArray(shape, dtype))
            zero_outs.append(np.zeros(shape, dtype))
    n_params = len(in_names)
    n_outs = len(out_avals)
    all_in_names = list(in_names) + list(out_names)
    if partition_name is not None:
        all_in_names.append(partition_name)

    def _body(*args):
        operands = list(args)
        if partition_name is not None:
            operands.append(bass2jax.partition_id_tensor())
        outs = bass2jax._bass_exec_p.bind(
            *operands,
            out_avals=tuple(out_avals),
            in_names=tuple(all_in_names),
            out_names=tuple(out_names),
            lowering_input_output_aliases=(),
            sim_require_finite=True,
            sim_require_nnan=True,
            nc=nc,
        )
        return tuple(outs)

    devices = jax.devices()[:N_CORES]
    assert len(devices) == N_CORES, f"need {N_CORES} cores, got {len(jax.devices())}"
    mesh = Mesh(np.asarray(devices), ("core",))
    in_specs = (PartitionSpec("core"),) * (n_params + n_outs)
    out_specs = (PartitionSpec("core"),) * n_outs
    donate = tuple(range(n_params, n_params + n_outs))
    sharded = jax.jit(
        shard_map(
            _body, mesh=mesh, in_specs=in_specs, out_specs=out_specs, check_rep=False
        ),
        donate_argnums=donate,
        keep_unused=True,
    )

    def run(in_maps):
        per_core = [[np.asarray(m[name]) for name in in_names] for m in in_maps]
        concat_in = [
            np.concatenate([per_core[c][i] for c in range(N_CORES)], axis=0)
            for i in range(n_params)
        ]
        concat_zeros = [
            np.zeros((N_CORES * z.shape[0], *z.shape[1:]), z.dtype) for z in zero_outs
        ]
        out_arrs = sharded(*concat_in, *concat_zeros)
        return [
            {
                name: np.asarray(out_arrs[i]).reshape(N_CORES, *out_avals[i].shape)[c]
                for i, name in enumerate(out_names)
            }
            for c in range(N_CORES)
        ]

    _RUNNER = run
    return _RUNNER


def _host_prep(inputs):
    emb = np.asarray(inputs["embedding"], np.float32)
    W_ih = np.asarray(inputs["W_ih"], np.float32)
    W_hh = np.asarray(inputs["W_hh"], np.float32)
    b_ih = np.asarray(inputs["b_ih"], np.float32)
    b_hh = np.asarray(inputs["b_hh"], np.float32)
    W_out = np.asarray(inputs["W_out"], np.float32)
    b_out = np.asarray(inputs["b_out"], np.float32)
    target = np.asarray(inputs["target_tensor"])
    h0 = np.asarray(inputs["encoder_hidden"], np.float32)[0]  # [B, H]

    # gate-column permutation: c = 768*jh + 256*g + f  <-  1024*g + 256*jh + f
    c = np.arange(G)
    jh, rem = c // 768, c % 768
    g, f = rem // 256, rem % 256
    perm = 1024 * g + 256 * jh + f

    bf = ml_dtypes.bfloat16
    wih_t = np.empty((H + 1, G), np.float32)
    wih_t[:H] = W_ih[perm].T
    rz_mask = (g < 2).astype(np.float32)
    bhh_p = b_hh[perm]
    wih_t[H] = b_ih[perm] + bhh_p * rz_mask
    whh_t = np.empty((H + 1, G), np.float32)
    whh_t[:H] = W_hh[perm].T
    whh_t[H] = bhh_p  # unused on device except bhn below
    # n-gate hidden bias, broadcast over batch in quad layout [32*jh+b, f]
    bhn = np.empty((128, 256), np.float32)
    for j in range(4):
        bhn[32 * j : 32 * (j + 1), :] = bhh_p[768 * j + 512 : 768 * j + 768][None, :]

    tokens = np.concatenate(
        [np.full((B, 1), SOS, target.dtype), target[:, : T - 1]], axis=1
    )  # [B, T]

    h0q = h0.reshape(B, 4, 256).transpose(1, 0, 2).reshape(128, 256)
    A = h0.reshape(B, 4, 2, 128)
    h0te = A[:, :, 0, :].transpose(2, 1, 0).reshape(128, 128)
    h0to = A[:, :, 1, :].transpose(2, 1, 0).reshape(128, 128)

    wih_b = wih_t.astype(bf)
    bhn_f = np.ascontiguousarray(bhn, np.float32)
    whh_b = whh_t.astype(bf)
    h0te_b = h0te.astype(bf)
    h0to_b = h0to.astype(bf)
    h0q_f = np.ascontiguousarray(h0q, np.float32)

    in_maps = []
    for cidx in range(N_CORES):
        ids = tokens[:, TPC * cidx : TPC * (cidx + 1)].T.reshape(-1)  # [256]
        X_c = emb[ids]  # [256, H]
        xt_c = np.ascontiguousarray(X_c.T).astype(bf)  # [H, 256]
        wo = np.empty((H + 1, VC), np.float32)
        wo[:H] = W_out[VC * cidx : VC * (cidx + 1)].T
        wo[H] = b_out[VC * cidx : VC * (cidx + 1)]
        in_maps.append(
            {
                "xt": xt_c,
                "wih": wih_b,
                "whh": whh_b,
                "wout": wo.astype(bf),
                "h0q": h0q_f,
                "h0te": h0te_b,
                "h0to": h0to_b,
                "bhn": bhn_f,
            }
        )
    return in_maps


def kernel(**inputs):
    in_maps = _host_prep(inputs)
    run = _get_runner()
    results = run(in_maps)

    shards = []
    for cidx in range(N_CORES):
        lp = results[cidx]["out_lp"]  # [NTAU, 128, VC]
        shards.append(
            lp.reshape(NTAU, 4, 32, VC).transpose(2, 0, 1, 3).reshape(B, T, VC)
        )
    log_probs = np.concatenate(shards, axis=2)  # [B, T, V]

    oh = results[0]["out_h"]  # [128, 256]
    h_final = oh.reshape(4, 32, 256).transpose(1, 0, 2).reshape(B, H)[None]

    return log_probs, h_final, None
